# revision 29
# baseline (speedup 1.0000x reference)
"""Two-layer GAT (GATConv x2 + LayerNorm + ELU + residual) on 8 trn2 NeuronCores.

Strategy (graph/data parallel, edge-cut by destination):
  - Nodes sharded contiguously: core k owns nodes [k*PN, (k+1)*PN).
  - Within a shard, nodes are sorted by in-degree and grouped into dst
    tiles of 128; dst node q of tile t is pinned to SBUF partition q.
    Chunk c of tile t holds the c-th in-edge of every dst in the tile
    (padded with masked slots up to the tile's max degree).
  - Per layer, each core computes rows [h(256) | es(4) | ed(4)] for its
    own nodes with one fused matmul (host-prepacked [W | W@A]); rows
    [h | es] are written in degree-sorted order and AllGathered into a
    full table.  Host pre-maps all gather indices through the
    permutation, so no scatters are needed anywhere.
  - Per chunk, an indirect DMA gathers the 128 per-edge source rows
    (one row per partition - the HW-native form).  p =
    mask * exp(leakyrelu(es[src] + ed[dst])) where ed broadcasts from a
    per-partition column (the dst IS the partition).  The segment-max
    subtraction cancels in the softmax and logits are tiny, so it is
    skipped.  Messages [p*h | p] accumulate over chunks into PSUM via
    identity matmuls; num/den divide, bias, residual, LayerNorm, ELU
    finish the tile.
  - Output shards are inverse-permuted and concatenated on the host.
"""

import math
import numpy as np
from contextlib import ExitStack

P = 128

N_FULL = 30000
E_FULL = 480000
IN_DIM = 128
HID = 256
HEADS = 4
C = HID // HEADS
NEG_SLOPE = 0.2
NCORES = 8
LN_EPS = 1e-5

# compute/table dtype: "f32" (exact, 4x slower matmuls) or "bf16"
# (table, gathers and matmuls in bf16; attention logits stay f32)
MM_DTYPE = "f16"


def _pack_att(att_src, att_dst):
    A = np.zeros((HID, 2 * HEADS), dtype=np.float32)
    for hd in range(HEADS):
        A[hd * C:(hd + 1) * C, hd] = att_src[hd]
        A[hd * C:(hd + 1) * C, HEADS + hd] = att_dst[hd]
    return A


def _prep_edges(edge_index, n, ncores):
    """Degree-sorted dst-per-partition edge layout.

    Returns (streams, perms, sched):
      streams[k] = (isrc, mask): [P, TOTCH] arrays.  isrc values are
        PERMUTED global row ids (k(src)*pn + permpos within shard).
      perms[k] = permutation of local node ids (degree-ascending).
      sched[t] = chunk count of dst tile t (shared across cores).
    """
    pn = n // ncores
    nt = (pn + P - 1) // P
    loop = np.arange(n, dtype=np.int64)
    src = np.concatenate([edge_index[0].astype(np.int64), loop])
    dst = np.concatenate([edge_index[1].astype(np.int64), loop])

    per_core = []
    perms = []
    permpos_g = np.empty(n, dtype=np.int64)  # global node -> permuted global row
    deg_all = []
    for k in range(ncores):
        sel = np.where((dst >= k * pn) & (dst < (k + 1) * pn))[0]
        s = src[sel]
        dl = (dst[sel] - k * pn)
        deg = np.bincount(dl, minlength=pn)
        perm = np.argsort(deg, kind="stable")          # local ids, deg ascending
        permpos = np.empty(pn, dtype=np.int64)
        permpos[perm] = np.arange(pn)
        permpos_g[k * pn:(k + 1) * pn] = k * pn + permpos
        perms.append(perm)
        deg_all.append(deg)
        per_core.append((s, dl, deg, permpos))

    # shared chunk schedule: per tile, max (over cores) of tile max degree
    sched = []
    for t in range(nt):
        mx = 1
        for k in range(ncores):
            deg, perm = deg_all[k], perms[k]
            q = perm[t * P:min((t + 1) * P, pn)]
            mx = max(mx, int(deg[q].max()) if len(q) else 1)
        sched.append(mx)
    offs = np.concatenate([[0], np.cumsum(sched)]).astype(np.int64)
    totch = int(offs[-1])

    streams = []
    for k in range(ncores):
        s, dl, deg, permpos = per_core[k]
        isrc = np.zeros((P, totch), dtype=np.int32)
        mask = np.zeros((P, totch), dtype=np.float32)
        order = np.argsort(dl, kind="stable")
        s_o = s[order]
        dl_o = dl[order]
        cstart = np.concatenate([[0], np.cumsum(deg)]).astype(np.int64)
        j = np.arange(len(s_o)) - cstart[dl_o]        # within-dst edge index
        q = permpos[dl_o]                              # permuted position of dst
        t_idx = q // P
        p_idx = q % P
        col = offs[t_idx] + j
        isrc[p_idx, col] = permpos_g[s_o].astype(np.int32)
        mask[p_idx, col] = 1.0
        streams.append((isrc, mask))
    return streams, perms, sched


def build_program(pn, n, sched, num_devices, mm_dtype=MM_DTYPE,
                  debug_dump=False):
    import concourse.bass as bass
    import concourse.bacc as bacc
    import concourse.tile as tile
    from concourse import mybir

    f32 = mybir.dt.float32
    i32 = mybir.dt.int32
    ALU = mybir.AluOpType
    ACT = mybir.ActivationFunctionType

    bf16 = mm_dtype in ("bf16", "f16")
    wdt = {"bf16": mybir.dt.bfloat16, "f16": mybir.dt.float16,
           "f32": f32}[mm_dtype]

    nt = (pn + P - 1) // P
    totch = int(sum(sched))
    maxch = int(max(sched))
    TW = 2 * HEADS
    # table row: f32 [h(256) | es(4)] = 260 f32
    # bf16      [h(256) bf16 | es(4) f32-as-8-bf16] = 264 bf16 units
    ROW = HID + (8 if bf16 else HEADS)
    DEN0 = HID          # den (p) columns start
    NMM = HID + HEADS   # matmul rhs width: num(256) + den(4)

    nc = bacc.Bacc("TRN2", target_bir_lowering=False, debug=False,
                   num_devices=num_devices)

    xT = nc.dram_tensor("xT", [IN_DIM, pn], wdt, kind="ExternalInput")
    w1cat_d = nc.dram_tensor("W1cat", [IN_DIM, HID + TW], wdt, kind="ExternalInput")
    wp_d = nc.dram_tensor("Wp", [IN_DIM, HID], wdt, kind="ExternalInput")
    w2cat_d = nc.dram_tensor("W2cat", [HID, HID + TW], wdt, kind="ExternalInput")
    consts_d = nc.dram_tensor("consts", [P, 7 * HID], f32, kind="ExternalInput")
    ident_d = nc.dram_tensor("ident", [P, P], f32, kind="ExternalInput")
    isrc_d = nc.dram_tensor("isrc", [P, totch], i32, kind="ExternalInput")
    mask_d = nc.dram_tensor("mask", [P, totch], f32, kind="ExternalInput")
    out_d = nc.dram_tensor("out_shard", [pn, HID], f32, kind="ExternalOutput")

    CB1, CG1, CBE1, CB2, CG2, CBE2, CBP = (i * HID for i in range(7))
    rg = [list(range(num_devices))]

    dbg = {}
    if debug_dump:
        dbg["table"] = nc.dram_tensor("dbg_table", [n, ROW], wdt,
                                      kind="ExternalOutput")
        dbg["v0"] = nc.dram_tensor("dbg_v0", [P, maxch * ROW], wdt,
                                   kind="ExternalOutput")
        dbg["p0"] = nc.dram_tensor("dbg_p0", [P, maxch * HEADS], f32,
                                   kind="ExternalOutput")
        dbg["po0"] = nc.dram_tensor("dbg_po0", [P, NMM], f32,
                                    kind="ExternalOutput")
        dbg["y1"] = nc.dram_tensor("dbg_y1", [P, nt * HID], f32,
                                   kind="ExternalOutput")

    with tile.TileContext(nc) as tc, ExitStack() as ctx:
        dram = ctx.enter_context(tc.tile_pool(name="dram", bufs=1, space="DRAM"))
        table_loc = [dram.tile([pn, ROW], wdt, name=f"table_loc{i}")
                     for i in range(2)]
        table_full = [dram.tile([n, ROW], wdt, name=f"table_full{i}",
                                addr_space="Shared")
                      for i in range(2)]
        ed_dram = [dram.tile([pn, HEADS], f32, name=f"ed_dram{i}")
                   for i in range(2)]
        res_dram = dram.tile([pn, HID], f32)

        singles = ctx.enter_context(tc.tile_pool(name="singles", bufs=1))
        persist = ctx.enter_context(tc.tile_pool(name="persist", bufs=1))
        nodeio = ctx.enter_context(tc.tile_pool(name="nodeio", bufs=2))
        edgeio = ctx.enter_context(tc.tile_pool(name="edgeio", bufs=2))
        small = ctx.enter_context(tc.tile_pool(name="small", bufs=3))
        psA = ctx.enter_context(tc.tile_pool(name="psA", bufs=2, space="PSUM"))
        psB = ctx.enter_context(tc.tile_pool(name="psB", bufs=2, space="PSUM"))
        psT = ctx.enter_context(tc.tile_pool(name="psT", bufs=2, space="PSUM"))

        w1cat = singles.tile([P, HID + TW], wdt)
        nc.sync.dma_start(out=w1cat[:], in_=w1cat_d[:])
        wp = singles.tile([P, HID], wdt)
        nc.sync.dma_start(out=wp[:], in_=wp_d[:])
        w2a = singles.tile([P, HID + TW], wdt)
        nc.sync.dma_start(out=w2a[:], in_=w2cat_d[0:P, :])
        w2b = singles.tile([P, HID + TW], wdt)
        nc.sync.dma_start(out=w2b[:], in_=w2cat_d[P:HID, :])
        consts = singles.tile([P, 7 * HID], f32)
        nc.sync.dma_start(out=consts[:], in_=consts_d[:])
        ident = singles.tile([P, P], f32)
        nc.sync.dma_start(out=ident[:], in_=ident_d[:])
        if bf16:
            identw = singles.tile([P, P], wdt)
            nc.vector.tensor_copy(out=identw[:], in_=ident[:])
        else:
            identw = ident
        epst = singles.tile([P, 1], f32)
        nc.vector.memset(epst[:], LN_EPS)

        y1 = persist.tile([P, nt * HID], f32)
        y1T = persist.tile([P, 2 * nt * P], wdt)

        def node_stage(layer):
            for t in range(nt):
                m = min(P, pn - t * P)
                ph = psA.tile([P, HID + TW], f32, tag="ph")
                if layer == 0:
                    lx = nodeio.tile([P, P], wdt, tag="lx")
                    if m < P:
                        nc.gpsimd.memset(lx[:, m:P], 0.0)
                    nc.sync.dma_start(out=lx[:, :m], in_=xT[:, t * P:t * P + m])
                    nc.tensor.matmul(ph[:], lhsT=lx[:], rhs=w1cat[:],
                                     start=True, stop=True)
                    pr = psA.tile([P, HID], f32, tag="pr")
                    nc.tensor.matmul(pr[:], lhsT=lx[:], rhs=wp[:],
                                     start=True, stop=True)
                    rsb = nodeio.tile([P, HID], f32, tag="rsb")
                    nc.vector.tensor_tensor(out=rsb[:], in0=pr[:],
                                            in1=consts[:, CBP:CBP + HID],
                                            op=ALU.add)
                    nc.sync.dma_start(out=res_dram[t * P:t * P + m, :],
                                      in_=rsb[:m, :])
                else:
                    nc.tensor.matmul(ph[:], lhsT=y1T[:, t * P:(t + 1) * P],
                                     rhs=w2a[:], start=True, stop=False)
                    nc.tensor.matmul(ph[:], lhsT=y1T[:, nt * P + t * P:
                                                     nt * P + (t + 1) * P],
                                     rhs=w2b[:], start=False, stop=True)
                if bf16:
                    hsb = nodeio.tile([P, HID + 16], wdt, tag="hsb")
                    nc.vector.tensor_copy(out=hsb[:, 0:HID], in_=ph[:, 0:HID])
                    nc.vector.tensor_copy(
                        out=hsb[:, HID:HID + 16].bitcast(f32),
                        in_=ph[:, HID:HID + TW])
                    nc.sync.dma_start(out=table_loc[layer][t * P:t * P + m, :],
                                      in_=hsb[:m, 0:ROW])
                    nc.sync.dma_start(
                        out=ed_dram[layer][t * P:t * P + m, :],
                        in_=hsb[:m, HID + 8:HID + 16].bitcast(f32))
                else:
                    hsb = nodeio.tile([P, HID + TW], f32, tag="hsb")
                    nc.vector.tensor_copy(out=hsb[:], in_=ph[:])
                    nc.sync.dma_start(out=table_loc[layer][t * P:t * P + m, :],
                                      in_=hsb[:m, 0:ROW])
                    nc.sync.dma_start(out=ed_dram[layer][t * P:t * P + m, :],
                                      in_=hsb[:m, ROW:ROW + HEADS])

        def agg_stage(layer):
            cb = CB1 if layer == 0 else CB2
            cg = CG1 if layer == 0 else CG2
            cbe = CBE1 if layer == 0 else CBE2
            off = 0
            for t in range(nt):
                nch = sched[t]
                m = min(P, pn - t * P)

                isrc = small.tile([P, maxch], i32, tag="isrc")
                nc.sync.dma_start(out=isrc[:, :nch], in_=isrc_d[:, off:off + nch])
                maskt = small.tile([P, maxch], f32, tag="maskt")
                nc.sync.dma_start(out=maskt[:, :nch], in_=mask_d[:, off:off + nch])
                edt = small.tile([P, HEADS], f32, tag="edt")
                if m < P:
                    nc.gpsimd.memset(edt[:], 0.0)
                nc.sync.dma_start(out=edt[:m, :],
                                  in_=ed_dram[layer][t * P:t * P + m, :])

                V = edgeio.tile([P, maxch * ROW], wdt, tag="V")
                for c in range(nch):
                    nc.gpsimd.indirect_dma_start(
                        out=V[:, c * ROW:(c + 1) * ROW], out_offset=None,
                        in_=table_full[layer][:],
                        in_offset=bass.IndirectOffsetOnAxis(
                            ap=isrc[:, c:c + 1], axis=0))
                V3 = V[:, :nch * ROW].rearrange("p (c d) -> p c d", d=ROW)
                if bf16:
                    es_view = V3[:, :, HID:HID + 8].bitcast(f32)
                else:
                    es_view = V3[:, :, HID:HID + HEADS]

                if debug_dump and layer == 0 and t == 0:
                    nc.sync.dma_start(out=dbg["v0"][:, :nch * ROW],
                                      in_=V[:, :nch * ROW])

                # p = mask * exp(leakyrelu(es + ed))
                e1 = small.tile([P, maxch * HEADS], f32, tag="e1")
                nc.vector.tensor_tensor(
                    out=e1[:, :nch * HEADS].rearrange("p (c d) -> p c d", d=HEADS),
                    in0=es_view,
                    in1=edt[:, None, :].to_broadcast([P, nch, HEADS]),
                    op=ALU.add)
                e2 = small.tile([P, maxch * HEADS], f32, tag="e2")
                nc.vector.tensor_scalar_mul(e2[:, :nch * HEADS],
                                            e1[:, :nch * HEADS], NEG_SLOPE)
                nc.vector.tensor_tensor(out=e2[:, :nch * HEADS],
                                        in0=e1[:, :nch * HEADS],
                                        in1=e2[:, :nch * HEADS], op=ALU.max)
                p1 = small.tile([P, maxch * HEADS], f32, tag="p1")
                nc.scalar.activation(out=p1[:, :nch * HEADS],
                                     in_=e2[:, :nch * HEADS], func=ACT.Exp)
                p3 = p1[:, :nch * HEADS].rearrange("p (c d) -> p c d", d=HEADS)
                nc.vector.tensor_tensor(
                    out=p3, in0=p3,
                    in1=maskt[:, :nch, None].to_broadcast([P, nch, HEADS]),
                    op=ALU.mult)
                if bf16:
                    p1w = small.tile([P, maxch * HEADS], wdt, tag="p1w")
                    nc.vector.tensor_copy(out=p1w[:, :nch * HEADS],
                                          in_=p1[:, :nch * HEADS])
                    p3w = p1w[:, :nch * HEADS].rearrange("p (c d) -> p c d",
                                                         d=HEADS)
                else:
                    p3w = p3

                # scale V in place: cols 0:256 *= p (per head), den cols = p
                for hd in range(HEADS):
                    nc.vector.tensor_tensor(
                        out=V3[:, :, hd * C:(hd + 1) * C],
                        in0=V3[:, :, hd * C:(hd + 1) * C],
                        in1=p3w[:, :, hd:hd + 1].to_broadcast([P, nch, C]),
                        op=ALU.mult)
                nc.vector.tensor_copy(out=V3[:, :, DEN0:DEN0 + HEADS], in_=p3w)

                if debug_dump and layer == 0 and t == 0:
                    nc.sync.dma_start(out=dbg["p0"][:, :nch * HEADS],
                                      in_=p1[:, :nch * HEADS])

                po = psB.tile([P, NMM], f32, tag="po")
                for c in range(nch):
                    nc.tensor.matmul(po[:], lhsT=identw[:],
                                     rhs=V[:, c * ROW:c * ROW + NMM],
                                     start=(c == 0), stop=(c == nch - 1))

                if debug_dump and layer == 0 and t == 0:
                    pod = nodeio.tile([P, NMM], f32, tag="pod")
                    nc.vector.tensor_copy(out=pod[:], in_=po[:])
                    nc.sync.dma_start(out=dbg["po0"][:], in_=pod[:])

                rec = small.tile([P, HEADS], f32, tag="rec")
                nc.vector.tensor_scalar_max(rec[:], po[:, HID:HID + HEADS], 1e-30)
                nc.vector.reciprocal(rec[:], rec[:])
                o = nodeio.tile([P, HID], f32, tag="o")
                for hd in range(HEADS):
                    nc.vector.tensor_tensor(
                        out=o[:, hd * C:(hd + 1) * C],
                        in0=po[:, hd * C:(hd + 1) * C],
                        in1=rec[:, hd:hd + 1].to_broadcast([P, C]),
                        op=ALU.mult)
                nc.vector.tensor_tensor(out=o[:], in0=o[:],
                                        in1=consts[:, cb:cb + HID], op=ALU.add)
                if layer == 0:
                    rsb = nodeio.tile([P, HID], f32, tag="rres")
                    if m < P:
                        nc.gpsimd.memset(rsb[:], 0.0)
                    nc.sync.dma_start(out=rsb[:m, :],
                                      in_=res_dram[t * P:t * P + m, :])
                    nc.vector.tensor_tensor(out=o[:], in0=o[:], in1=rsb[:],
                                            op=ALU.add)
                else:
                    nc.vector.tensor_tensor(out=o[:], in0=o[:],
                                            in1=y1[:, t * HID:(t + 1) * HID],
                                            op=ALU.add)

                stats = small.tile([P, 6], f32, tag="stats")
                nc.vector.bn_stats(out=stats[:], in_=o[:])
                mv = small.tile([P, 2], f32, tag="mv")
                nc.vector.bn_aggr(out=mv[:], in_=stats[:])
                nc.scalar.activation(out=mv[:, 1:2], in_=mv[:, 1:2],
                                     func=ACT.Sqrt, bias=epst[:], scale=1.0)
                nc.vector.reciprocal(mv[:, 1:2], mv[:, 1:2])
                yn = nodeio.tile([P, HID], f32, tag="yn")
                nc.vector.tensor_scalar(yn[:], o[:], mv[:, 0:1], mv[:, 1:2],
                                        op0=ALU.subtract, op1=ALU.mult)
                nc.vector.tensor_tensor(out=yn[:], in0=yn[:],
                                        in1=consts[:, cg:cg + HID], op=ALU.mult)
                nc.vector.tensor_tensor(out=yn[:], in0=yn[:],
                                        in1=consts[:, cbe:cbe + HID], op=ALU.add)

                # ELU = (max(y,0) - 1) + exp(min(y,0))
                t0 = nodeio.tile([P, HID], f32, tag="t0")
                nc.vector.tensor_scalar_min(t0[:], yn[:], 0.0)
                t1 = nodeio.tile([P, HID], f32, tag="t1")
                nc.scalar.activation(out=t1[:], in_=t0[:], func=ACT.Exp)
                t2 = nodeio.tile([P, HID], f32, tag="t2")
                nc.vector.tensor_scalar(t2[:], yn[:], 0.0, -1.0,
                                        op0=ALU.max, op1=ALU.add)

                if layer == 0:
                    nc.vector.tensor_tensor(out=y1[:, t * HID:(t + 1) * HID],
                                            in0=t1[:], in1=t2[:], op=ALU.add)
                    for h in range(2):
                        pt = psT.tile([P, P], f32, tag="pt")
                        nc.tensor.transpose(
                            out=pt[:],
                            in_=y1[:, t * HID + h * P:t * HID + (h + 1) * P],
                            identity=ident[:])
                        nc.vector.tensor_copy(
                            out=y1T[:, h * nt * P + t * P:
                                    h * nt * P + (t + 1) * P],
                            in_=pt[:])
                else:
                    fo = nodeio.tile([P, HID], f32, tag="fo")
                    nc.vector.tensor_tensor(out=fo[:], in0=t1[:], in1=t2[:],
                                            op=ALU.add)
                    nc.sync.dma_start(out=out_d[t * P:t * P + m, :],
                                      in_=fo[:m, :])
                off += nch

        from concourse import mybir as _mb
        for layer in range(2):
            node_stage(layer)
            nc.gpsimd.collective_compute(
                "AllGather", _mb.AluOpType.bypass,
                ins=[table_loc[layer][:].opt()],
                outs=[table_full[layer][:].opt()],
                replica_groups=rg)
            if debug_dump and layer == 0:
                nc.sync.dma_start(out=dbg["table"][:], in_=table_full[0][:])
            agg_stage(layer)
            if debug_dump and layer == 0:
                nc.sync.dma_start(out=dbg["y1"][:], in_=y1[:])

    nc.compile()
    return nc


def make_in_maps(inputs, n, ncores):
    """Host-side sharding: returns (in_maps, perms, sched, pn)."""
    x = np.asarray(inputs["x"], dtype=np.float32)
    edge_index = np.asarray(inputs["edge_index"])
    pn = n // ncores

    streams, perms, sched = _prep_edges(edge_index, n, ncores)

    A1 = _pack_att(np.asarray(inputs["att_src1"], np.float32),
                   np.asarray(inputs["att_dst1"], np.float32))
    A2 = _pack_att(np.asarray(inputs["att_src2"], np.float32),
                   np.asarray(inputs["att_dst2"], np.float32))
    W1 = np.asarray(inputs["W1"], np.float32)
    W2 = np.asarray(inputs["W2"], np.float32)
    W1cat = np.hstack([W1, W1 @ A1]).astype(np.float32)
    W2cat = np.hstack([W2, W2 @ A2]).astype(np.float32)
    Wp = np.asarray(inputs["Wp"], np.float32)

    crow = np.concatenate([
        np.asarray(inputs["b1"], np.float32),
        np.asarray(inputs["g1"], np.float32),
        np.asarray(inputs["be1"], np.float32),
        np.asarray(inputs["b2"], np.float32),
        np.asarray(inputs["g2"], np.float32),
        np.asarray(inputs["be2"], np.float32),
        np.asarray(inputs["bp"], np.float32),
    ])
    consts = np.tile(crow[None, :], (P, 1)).astype(np.float32)
    ident = np.eye(P, dtype=np.float32)

    if MM_DTYPE in ("bf16", "f16"):
        import ml_dtypes
        bf = ml_dtypes.bfloat16 if MM_DTYPE == "bf16" else np.float16
        W1cat = W1cat.astype(bf)
        W2cat = W2cat.astype(bf)
        Wp = Wp.astype(bf)
        x = x.astype(bf)

    in_maps = []
    for k in range(ncores):
        isrc, mask = streams[k]
        xp = x[k * pn + perms[k]]           # permuted shard rows
        in_maps.append({
            "xT": np.ascontiguousarray(xp.T),
            "W1cat": W1cat, "Wp": Wp, "W2cat": W2cat,
            "consts": consts, "ident": ident,
            "isrc": isrc, "mask": mask,
        })
    return in_maps, perms, sched, pn


def assemble_out(res_list, perms, pn):
    outs = []
    for k, perm in enumerate(perms):
        o = res_list[k]
        inv = np.empty_like(perm)
        inv[perm] = np.arange(pn)
        outs.append(o[inv])
    return np.concatenate(outs, axis=0)


def kernel(**inputs):
    from concourse.bass_utils import run_bass_kernel_spmd

    in_maps, perms, sched, pn = make_in_maps(inputs, N_FULL, NCORES)
    nc = build_program(pn, N_FULL, sched, NCORES)
    res = run_bass_kernel_spmd(nc, in_maps, list(range(NCORES)))
    out = assemble_out([res.results[k]["out_shard"] for k in range(NCORES)],
                       perms, pn)
    return out.astype(np.float32)


# revision 32
# speedup vs baseline: 1.0452x; 1.0452x over previous
"""Two-layer GAT (GATConv x2 + LayerNorm + ELU + residual) on 8 trn2 NeuronCores.

Strategy (graph/data parallel, edge-cut by destination):
  - Nodes sharded contiguously: core k owns nodes [k*PN, (k+1)*PN).
  - Within a shard, nodes are sorted by in-degree and grouped into dst
    tiles of 128; dst node q of tile t is pinned to SBUF partition q.
    Chunk c of tile t holds the c-th in-edge of every dst in the tile
    (padded with masked slots up to the tile's max degree).
  - Per layer, each core computes rows [h(256) | es(4) | ed(4)] for its
    own nodes with one fused matmul (host-prepacked [W | W@A]); rows
    [h | es] are written in degree-sorted order and AllGathered into a
    full table.  Host pre-maps all gather indices through the
    permutation, so no scatters are needed anywhere.
  - Per chunk, an indirect DMA gathers the 128 per-edge source rows
    (one row per partition - the HW-native form).  p =
    mask * exp(leakyrelu(es[src] + ed[dst])) where ed broadcasts from a
    per-partition column (the dst IS the partition).  The segment-max
    subtraction cancels in the softmax and logits are tiny, so it is
    skipped.  Messages [p*h | p] accumulate over chunks into PSUM via
    identity matmuls; num/den divide, bias, residual, LayerNorm, ELU
    finish the tile.
  - Output shards are inverse-permuted and concatenated on the host.
"""

import math
import numpy as np
from contextlib import ExitStack

P = 128

N_FULL = 30000
E_FULL = 480000
IN_DIM = 128
HID = 256
HEADS = 4
C = HID // HEADS
NEG_SLOPE = 0.2
NCORES = 8
LN_EPS = 1e-5

# compute/table dtype: "f32" (exact, 4x slower matmuls) or "bf16"
# (table, gathers and matmuls in bf16; attention logits stay f32)
MM_DTYPE = "f16"


def _pack_att(att_src, att_dst):
    A = np.zeros((HID, 2 * HEADS), dtype=np.float32)
    for hd in range(HEADS):
        A[hd * C:(hd + 1) * C, hd] = att_src[hd]
        A[hd * C:(hd + 1) * C, HEADS + hd] = att_dst[hd]
    return A


def _prep_edges(edge_index, n, ncores):
    """Degree-sorted dst-per-partition edge layout.

    Returns (streams, perms, sched):
      streams[k] = (isrc, mask): [P, TOTCH] arrays.  isrc values are
        PERMUTED global row ids (k(src)*pn + permpos within shard).
      perms[k] = permutation of local node ids (degree-ascending).
      sched[t] = chunk count of dst tile t (shared across cores).
    """
    pn = n // ncores
    nt = (pn + P - 1) // P
    loop = np.arange(n, dtype=np.int64)
    src = np.concatenate([edge_index[0].astype(np.int64), loop])
    dst = np.concatenate([edge_index[1].astype(np.int64), loop])

    per_core = []
    perms = []
    permpos_g = np.empty(n, dtype=np.int64)  # global node -> permuted global row
    deg_all = []
    for k in range(ncores):
        sel = np.where((dst >= k * pn) & (dst < (k + 1) * pn))[0]
        s = src[sel]
        dl = (dst[sel] - k * pn)
        deg = np.bincount(dl, minlength=pn)
        perm = np.argsort(deg, kind="stable")          # local ids, deg ascending
        permpos = np.empty(pn, dtype=np.int64)
        permpos[perm] = np.arange(pn)
        permpos_g[k * pn:(k + 1) * pn] = k * pn + permpos
        perms.append(perm)
        deg_all.append(deg)
        per_core.append((s, dl, deg, permpos))

    # shared chunk schedule: per tile, max (over cores) of tile max degree
    sched = []
    for t in range(nt):
        mx = 1
        for k in range(ncores):
            deg, perm = deg_all[k], perms[k]
            q = perm[t * P:min((t + 1) * P, pn)]
            mx = max(mx, int(deg[q].max()) if len(q) else 1)
        sched.append(mx)
    offs = np.concatenate([[0], np.cumsum(sched)]).astype(np.int64)
    totch = int(offs[-1])

    streams = []
    for k in range(ncores):
        s, dl, deg, permpos = per_core[k]
        isrc = np.zeros((P, totch), dtype=np.int32)
        mask = np.zeros((P, totch), dtype=np.float32)
        order = np.argsort(dl, kind="stable")
        s_o = s[order]
        dl_o = dl[order]
        cstart = np.concatenate([[0], np.cumsum(deg)]).astype(np.int64)
        j = np.arange(len(s_o)) - cstart[dl_o]        # within-dst edge index
        q = permpos[dl_o]                              # permuted position of dst
        t_idx = q // P
        p_idx = q % P
        col = offs[t_idx] + j
        isrc[p_idx, col] = permpos_g[s_o].astype(np.int32)
        mask[p_idx, col] = 1.0
        streams.append((isrc, mask))
    return streams, perms, sched


def build_program(pn, n, sched, num_devices, mm_dtype=MM_DTYPE,
                  debug_dump=False):
    import concourse.bass as bass
    import concourse.bacc as bacc
    import concourse.tile as tile
    from concourse import mybir

    f32 = mybir.dt.float32
    i32 = mybir.dt.int32
    ALU = mybir.AluOpType
    ACT = mybir.ActivationFunctionType

    bf16 = mm_dtype in ("bf16", "f16")
    wdt = {"bf16": mybir.dt.bfloat16, "f16": mybir.dt.float16,
           "f32": f32}[mm_dtype]

    nt = (pn + P - 1) // P
    totch = int(sum(sched))
    maxch = int(max(sched))
    TW = 2 * HEADS
    # table row: f32 [h(256) | es(4)] = 260 f32
    # bf16      [h(256) bf16 | es(4) f32-as-8-bf16] = 264 bf16 units
    ROW = HID + (8 if bf16 else HEADS)
    DEN0 = HID          # den (p) columns start
    NMM = HID + HEADS   # matmul rhs width: num(256) + den(4)

    nc = bacc.Bacc("TRN2", target_bir_lowering=False, debug=False,
                   num_devices=num_devices)

    xT = nc.dram_tensor("xT", [IN_DIM, pn], wdt, kind="ExternalInput")
    w1cat_d = nc.dram_tensor("W1cat", [IN_DIM, HID + TW], wdt, kind="ExternalInput")
    wp_d = nc.dram_tensor("Wp", [IN_DIM, HID], wdt, kind="ExternalInput")
    w2cat_d = nc.dram_tensor("W2cat", [HID, HID + TW], wdt, kind="ExternalInput")
    consts_d = nc.dram_tensor("consts", [P, 7 * HID], f32, kind="ExternalInput")
    ident_d = nc.dram_tensor("ident", [P, P], f32, kind="ExternalInput")
    isrc_d = nc.dram_tensor("isrc", [P, totch], i32, kind="ExternalInput")
    mask_d = nc.dram_tensor("mask", [P, totch], f32, kind="ExternalInput")
    out_d = nc.dram_tensor("out_shard", [pn, HID], f32, kind="ExternalOutput")

    CB1, CG1, CBE1, CB2, CG2, CBE2, CBP = (i * HID for i in range(7))
    rg = [list(range(num_devices))]

    dbg = {}
    if debug_dump:
        dbg["table"] = nc.dram_tensor("dbg_table", [n, ROW], wdt,
                                      kind="ExternalOutput")
        dbg["v0"] = nc.dram_tensor("dbg_v0", [P, maxch * ROW], wdt,
                                   kind="ExternalOutput")
        dbg["p0"] = nc.dram_tensor("dbg_p0", [P, maxch * HEADS], f32,
                                   kind="ExternalOutput")
        dbg["po0"] = nc.dram_tensor("dbg_po0", [P, NMM], f32,
                                    kind="ExternalOutput")
        dbg["y1"] = nc.dram_tensor("dbg_y1", [P, nt * HID], f32,
                                   kind="ExternalOutput")

    with tile.TileContext(nc) as tc, ExitStack() as ctx:
        dram = ctx.enter_context(tc.tile_pool(name="dram", bufs=1, space="DRAM"))
        table_loc = [dram.tile([pn, ROW], wdt, name=f"table_loc{i}")
                     for i in range(2)]
        table_full = [dram.tile([n, ROW], wdt, name=f"table_full{i}",
                                addr_space="Shared")
                      for i in range(2)]
        ed_dram = [dram.tile([pn, HEADS], f32, name=f"ed_dram{i}")
                   for i in range(2)]

        singles = ctx.enter_context(tc.tile_pool(name="singles", bufs=1))
        persist = ctx.enter_context(tc.tile_pool(name="persist", bufs=1))
        nodeio = ctx.enter_context(tc.tile_pool(name="nodeio", bufs=2))
        edgeio = ctx.enter_context(tc.tile_pool(name="edgeio", bufs=2))
        small = ctx.enter_context(tc.tile_pool(name="small", bufs=3))
        psA = ctx.enter_context(tc.tile_pool(name="psA", bufs=2, space="PSUM"))
        psB = ctx.enter_context(tc.tile_pool(name="psB", bufs=3, space="PSUM"))
        psT = ctx.enter_context(tc.tile_pool(name="psT", bufs=2, space="PSUM"))

        w1cat = singles.tile([P, HID + TW], wdt)
        nc.sync.dma_start(out=w1cat[:], in_=w1cat_d[:])
        wp = singles.tile([P, HID], wdt)
        nc.sync.dma_start(out=wp[:], in_=wp_d[:])
        w2a = singles.tile([P, HID + TW], wdt)
        nc.sync.dma_start(out=w2a[:], in_=w2cat_d[0:P, :])
        w2b = singles.tile([P, HID + TW], wdt)
        nc.sync.dma_start(out=w2b[:], in_=w2cat_d[P:HID, :])
        consts = singles.tile([P, 7 * HID], f32)
        nc.sync.dma_start(out=consts[:], in_=consts_d[:])
        ident = singles.tile([P, P], f32)
        nc.sync.dma_start(out=ident[:], in_=ident_d[:])
        if bf16:
            identw = singles.tile([P, P], wdt)
            nc.vector.tensor_copy(out=identw[:], in_=ident[:])
        else:
            identw = ident
        epst = singles.tile([P, 1], f32)
        nc.vector.memset(epst[:], LN_EPS)

        y1 = persist.tile([P, nt * HID], f32)
        y1T = persist.tile([P, 2 * nt * P], wdt)
        res_sb = persist.tile([P, nt * HID], f32)

        def node_stage(layer):
            for t in range(nt):
                m = min(P, pn - t * P)
                ph = psA.tile([P, HID + TW], f32, tag="ph")
                if layer == 0:
                    lx = nodeio.tile([P, P], wdt, tag="lx")
                    if m < P:
                        nc.gpsimd.memset(lx[:, m:P], 0.0)
                    nc.sync.dma_start(out=lx[:, :m], in_=xT[:, t * P:t * P + m])
                    nc.tensor.matmul(ph[:], lhsT=lx[:], rhs=w1cat[:],
                                     start=True, stop=True)
                    pr = psA.tile([P, HID], f32, tag="pr")
                    nc.tensor.matmul(pr[:], lhsT=lx[:], rhs=wp[:],
                                     start=True, stop=True)
                    rsb = nodeio.tile([P, HID], f32, tag="rsb")
                    nc.vector.tensor_tensor(out=rsb[:], in0=pr[:],
                                            in1=consts[:, CBP:CBP + HID],
                                            op=ALU.add)
                    nc.sync.dma_start(out=res_dram[t * P:t * P + m, :],
                                      in_=rsb[:m, :])
                else:
                    nc.tensor.matmul(ph[:], lhsT=y1T[:, t * P:(t + 1) * P],
                                     rhs=w2a[:], start=True, stop=False)
                    nc.tensor.matmul(ph[:], lhsT=y1T[:, nt * P + t * P:
                                                     nt * P + (t + 1) * P],
                                     rhs=w2b[:], start=False, stop=True)
                if bf16:
                    hsb = nodeio.tile([P, HID + 16], wdt, tag="hsb")
                    nc.vector.tensor_copy(out=hsb[:, 0:HID], in_=ph[:, 0:HID])
                    nc.vector.tensor_copy(
                        out=hsb[:, HID:HID + 16].bitcast(f32),
                        in_=ph[:, HID:HID + TW])
                    nc.sync.dma_start(out=table_loc[layer][t * P:t * P + m, :],
                                      in_=hsb[:m, 0:ROW])
                    nc.sync.dma_start(
                        out=ed_dram[layer][t * P:t * P + m, :],
                        in_=hsb[:m, HID + 8:HID + 16].bitcast(f32))
                else:
                    hsb = nodeio.tile([P, HID + TW], f32, tag="hsb")
                    nc.vector.tensor_copy(out=hsb[:], in_=ph[:])
                    nc.sync.dma_start(out=table_loc[layer][t * P:t * P + m, :],
                                      in_=hsb[:m, 0:ROW])
                    nc.sync.dma_start(out=ed_dram[layer][t * P:t * P + m, :],
                                      in_=hsb[:m, ROW:ROW + HEADS])

        def agg_stage(layer):
            cb = CB1 if layer == 0 else CB2
            cg = CG1 if layer == 0 else CG2
            cbe = CBE1 if layer == 0 else CBE2
            off = 0
            for t in range(nt):
                nch = sched[t]
                m = min(P, pn - t * P)

                isrc = small.tile([P, maxch], i32, tag="isrc")
                nc.sync.dma_start(out=isrc[:, :nch], in_=isrc_d[:, off:off + nch])
                maskt = small.tile([P, maxch], f32, tag="maskt")
                nc.sync.dma_start(out=maskt[:, :nch], in_=mask_d[:, off:off + nch])
                edt = small.tile([P, HEADS], f32, tag="edt")
                if m < P:
                    nc.gpsimd.memset(edt[:], 0.0)
                nc.sync.dma_start(out=edt[:m, :],
                                  in_=ed_dram[layer][t * P:t * P + m, :])

                V = edgeio.tile([P, maxch * ROW], wdt, tag="V")
                for c in range(nch):
                    nc.gpsimd.indirect_dma_start(
                        out=V[:, c * ROW:(c + 1) * ROW], out_offset=None,
                        in_=table_full[layer][:],
                        in_offset=bass.IndirectOffsetOnAxis(
                            ap=isrc[:, c:c + 1], axis=0))
                V3 = V[:, :nch * ROW].rearrange("p (c d) -> p c d", d=ROW)
                if bf16:
                    es_view = V3[:, :, HID:HID + 8].bitcast(f32)
                else:
                    es_view = V3[:, :, HID:HID + HEADS]

                if debug_dump and layer == 0 and t == 0:
                    nc.sync.dma_start(out=dbg["v0"][:, :nch * ROW],
                                      in_=V[:, :nch * ROW])

                # p = mask * exp(leakyrelu(es + ed))
                e1 = small.tile([P, maxch * HEADS], f32, tag="e1")
                nc.vector.tensor_tensor(
                    out=e1[:, :nch * HEADS].rearrange("p (c d) -> p c d", d=HEADS),
                    in0=es_view,
                    in1=edt[:, None, :].to_broadcast([P, nch, HEADS]),
                    op=ALU.add)
                e2 = small.tile([P, maxch * HEADS], f32, tag="e2")
                nc.vector.tensor_scalar_mul(e2[:, :nch * HEADS],
                                            e1[:, :nch * HEADS], NEG_SLOPE)
                nc.vector.tensor_tensor(out=e2[:, :nch * HEADS],
                                        in0=e1[:, :nch * HEADS],
                                        in1=e2[:, :nch * HEADS], op=ALU.max)
                p1 = small.tile([P, maxch * HEADS], f32, tag="p1")
                nc.scalar.activation(out=p1[:, :nch * HEADS],
                                     in_=e2[:, :nch * HEADS], func=ACT.Exp)
                p3 = p1[:, :nch * HEADS].rearrange("p (c d) -> p c d", d=HEADS)
                nc.vector.tensor_tensor(
                    out=p3, in0=p3,
                    in1=maskt[:, :nch, None].to_broadcast([P, nch, HEADS]),
                    op=ALU.mult)
                if bf16:
                    p1w = small.tile([P, maxch * HEADS], wdt, tag="p1w")
                    nc.vector.tensor_copy(out=p1w[:, :nch * HEADS],
                                          in_=p1[:, :nch * HEADS])
                    p3w = p1w[:, :nch * HEADS].rearrange("p (c d) -> p c d",
                                                         d=HEADS)
                else:
                    p3w = p3

                # scale V in place: cols 0:256 *= p (per head), den cols = p
                for hd in range(HEADS):
                    nc.vector.tensor_tensor(
                        out=V3[:, :, hd * C:(hd + 1) * C],
                        in0=V3[:, :, hd * C:(hd + 1) * C],
                        in1=p3w[:, :, hd:hd + 1].to_broadcast([P, nch, C]),
                        op=ALU.mult)
                nc.vector.tensor_copy(out=V3[:, :, DEN0:DEN0 + HEADS], in_=p3w)

                if debug_dump and layer == 0 and t == 0:
                    nc.sync.dma_start(out=dbg["p0"][:, :nch * HEADS],
                                      in_=p1[:, :nch * HEADS])

                po = psB.tile([P, NMM], f32, tag="po")
                for c in range(nch):
                    nc.tensor.matmul(po[:], lhsT=identw[:],
                                     rhs=V[:, c * ROW:c * ROW + NMM],
                                     start=(c == 0), stop=(c == nch - 1))

                if debug_dump and layer == 0 and t == 0:
                    pod = nodeio.tile([P, NMM], f32, tag="pod")
                    nc.vector.tensor_copy(out=pod[:], in_=po[:])
                    nc.sync.dma_start(out=dbg["po0"][:], in_=pod[:])

                rec = small.tile([P, HEADS], f32, tag="rec")
                nc.vector.tensor_scalar_max(rec[:], po[:, HID:HID + HEADS], 1e-30)
                nc.vector.reciprocal(rec[:], rec[:])
                o = nodeio.tile([P, HID], f32, tag="o")
                for hd in range(HEADS):
                    nc.vector.tensor_tensor(
                        out=o[:, hd * C:(hd + 1) * C],
                        in0=po[:, hd * C:(hd + 1) * C],
                        in1=rec[:, hd:hd + 1].to_broadcast([P, C]),
                        op=ALU.mult)
                nc.vector.tensor_tensor(out=o[:], in0=o[:],
                                        in1=consts[:, cb:cb + HID], op=ALU.add)
                if layer == 0:
                    rsb = nodeio.tile([P, HID], f32, tag="rres")
                    if m < P:
                        nc.gpsimd.memset(rsb[:], 0.0)
                    nc.sync.dma_start(out=rsb[:m, :],
                                      in_=res_dram[t * P:t * P + m, :])
                    nc.vector.tensor_tensor(out=o[:], in0=o[:], in1=rsb[:],
                                            op=ALU.add)
                else:
                    nc.vector.tensor_tensor(out=o[:], in0=o[:],
                                            in1=y1[:, t * HID:(t + 1) * HID],
                                            op=ALU.add)

                stats = small.tile([P, 6], f32, tag="stats")
                nc.vector.bn_stats(out=stats[:], in_=o[:])
                mv = small.tile([P, 2], f32, tag="mv")
                nc.vector.bn_aggr(out=mv[:], in_=stats[:])
                nc.scalar.activation(out=mv[:, 1:2], in_=mv[:, 1:2],
                                     func=ACT.Sqrt, bias=epst[:], scale=1.0)
                nc.vector.reciprocal(mv[:, 1:2], mv[:, 1:2])
                yn = nodeio.tile([P, HID], f32, tag="yn")
                nc.vector.tensor_scalar(yn[:], o[:], mv[:, 0:1], mv[:, 1:2],
                                        op0=ALU.subtract, op1=ALU.mult)
                nc.vector.tensor_tensor(out=yn[:], in0=yn[:],
                                        in1=consts[:, cg:cg + HID], op=ALU.mult)
                nc.vector.tensor_tensor(out=yn[:], in0=yn[:],
                                        in1=consts[:, cbe:cbe + HID], op=ALU.add)

                # ELU = (max(y,0) - 1) + exp(min(y,0))
                t0 = nodeio.tile([P, HID], f32, tag="t0")
                nc.vector.tensor_scalar_min(t0[:], yn[:], 0.0)
                t1 = nodeio.tile([P, HID], f32, tag="t1")
                nc.scalar.activation(out=t1[:], in_=t0[:], func=ACT.Exp)
                t2 = nodeio.tile([P, HID], f32, tag="t2")
                nc.vector.tensor_scalar(t2[:], yn[:], 0.0, -1.0,
                                        op0=ALU.max, op1=ALU.add)

                if layer == 0:
                    nc.vector.tensor_tensor(out=y1[:, t * HID:(t + 1) * HID],
                                            in0=t1[:], in1=t2[:], op=ALU.add)
                    for h in range(2):
                        pt = psT.tile([P, P], f32, tag="pt")
                        nc.tensor.transpose(
                            out=pt[:],
                            in_=y1[:, t * HID + h * P:t * HID + (h + 1) * P],
                            identity=ident[:])
                        nc.vector.tensor_copy(
                            out=y1T[:, h * nt * P + t * P:
                                    h * nt * P + (t + 1) * P],
                            in_=pt[:])
                else:
                    fo = nodeio.tile([P, HID], f32, tag="fo")
                    nc.vector.tensor_tensor(out=fo[:], in0=t1[:], in1=t2[:],
                                            op=ALU.add)
                    nc.sync.dma_start(out=out_d[t * P:t * P + m, :],
                                      in_=fo[:m, :])
                off += nch

        from concourse import mybir as _mb
        for layer in range(2):
            node_stage(layer)
            nc.gpsimd.collective_compute(
                "AllGather", _mb.AluOpType.bypass,
                ins=[table_loc[layer][:].opt()],
                outs=[table_full[layer][:].opt()],
                replica_groups=rg)
            if debug_dump and layer == 0:
                nc.sync.dma_start(out=dbg["table"][:], in_=table_full[0][:])
            agg_stage(layer)
            if debug_dump and layer == 0:
                nc.sync.dma_start(out=dbg["y1"][:], in_=y1[:])

    nc.compile()
    return nc


def make_in_maps(inputs, n, ncores):
    """Host-side sharding: returns (in_maps, perms, sched, pn)."""
    x = np.asarray(inputs["x"], dtype=np.float32)
    edge_index = np.asarray(inputs["edge_index"])
    pn = n // ncores

    streams, perms, sched = _prep_edges(edge_index, n, ncores)

    A1 = _pack_att(np.asarray(inputs["att_src1"], np.float32),
                   np.asarray(inputs["att_dst1"], np.float32))
    A2 = _pack_att(np.asarray(inputs["att_src2"], np.float32),
                   np.asarray(inputs["att_dst2"], np.float32))
    W1 = np.asarray(inputs["W1"], np.float32)
    W2 = np.asarray(inputs["W2"], np.float32)
    W1cat = np.hstack([W1, W1 @ A1]).astype(np.float32)
    W2cat = np.hstack([W2, W2 @ A2]).astype(np.float32)
    Wp = np.asarray(inputs["Wp"], np.float32)

    crow = np.concatenate([
        np.asarray(inputs["b1"], np.float32),
        np.asarray(inputs["g1"], np.float32),
        np.asarray(inputs["be1"], np.float32),
        np.asarray(inputs["b2"], np.float32),
        np.asarray(inputs["g2"], np.float32),
        np.asarray(inputs["be2"], np.float32),
        np.asarray(inputs["bp"], np.float32),
    ])
    consts = np.tile(crow[None, :], (P, 1)).astype(np.float32)
    ident = np.eye(P, dtype=np.float32)

    if MM_DTYPE in ("bf16", "f16"):
        import ml_dtypes
        bf = ml_dtypes.bfloat16 if MM_DTYPE == "bf16" else np.float16
        W1cat = W1cat.astype(bf)
        W2cat = W2cat.astype(bf)
        Wp = Wp.astype(bf)
        x = x.astype(bf)

    in_maps = []
    for k in range(ncores):
        isrc, mask = streams[k]
        xp = x[k * pn + perms[k]]           # permuted shard rows
        in_maps.append({
            "xT": np.ascontiguousarray(xp.T),
            "W1cat": W1cat, "Wp": Wp, "W2cat": W2cat,
            "consts": consts, "ident": ident,
            "isrc": isrc, "mask": mask,
        })
    return in_maps, perms, sched, pn


def assemble_out(res_list, perms, pn):
    outs = []
    for k, perm in enumerate(perms):
        o = res_list[k]
        inv = np.empty_like(perm)
        inv[perm] = np.arange(pn)
        outs.append(o[inv])
    return np.concatenate(outs, axis=0)


def kernel(**inputs):
    from concourse.bass_utils import run_bass_kernel_spmd

    in_maps, perms, sched, pn = make_in_maps(inputs, N_FULL, NCORES)
    nc = build_program(pn, N_FULL, sched, NCORES)
    res = run_bass_kernel_spmd(nc, in_maps, list(range(NCORES)))
    out = assemble_out([res.results[k]["out_shard"] for k in range(NCORES)],
                       perms, pn)
    return out.astype(np.float32)
